# revision 23
# baseline (speedup 1.0000x reference)
"""Trainium2 Bass kernel for nn_CustomModel_31361851195525.

Reference computation (b=4, s=4096, d=256):
    x0 = rope(x @ W0)[:, 0, :]          # rope at pos 0 == identity
    x1 = rope(x @ W1)                   # (b, s, d)
    scores[b, t] = x0[b] . x1[b, t]     # only row 0 of the score matrix is used
    attn = softmax(scores)              # (b, s)
    out[b] = sum_t attn[b, t] * (x @ W2)[b, t]
           = (attn[b] @ x[b]) @ W2      # reassociated: kills the big x@W2 matmul

Score folding (avoids materializing rope(x@W1)):
    scores[t] = sum_k cos[t,k] * (x_t . A_k) + sin[t,k] * (x_t . B_k)
    A[:, k] = W1[:, 2k] * x0[2k]   + W1[:, 2k+1] * x0[2k+1]
    B[:, k] = W1[:, 2k] * x0[2k+1] - W1[:, 2k+1] * x0[2k]

Sharding: core c -> batch c//2, sequence half c%2 (2048 rows each).
Each core returns (v, m, s) = (unnormalized weighted x-sum @ W2, chunk score
max, chunk exp-sum); the host combines the two chunks per batch with the
standard log-sum-exp merge.
"""

import math

import numpy as np

SEQ = 4096
DIM = 256
HALF = 128
NB = 4
NCORES = 8
CHUNK = SEQ // 2       # rows per core
NJ = CHUNK // 128      # 16 row tiles per core
NCH = CHUNK // 512     # 4 score chunks per core

# "host": sin/cos tables computed on host and DMA'd in (2MB extra per core).
# "device": tables built on-device via iota + mod range-reduction + Sin LUT.
TABLE_MODE = "device"

_CACHE = {}


def _build_nc(table_mode):
    import concourse.bass as bass
    import concourse.mybir as mybir
    import concourse.tile as tile
    from concourse import bacc
    from concourse.masks import make_identity

    fp32 = mybir.dt.float32
    AF = mybir.ActivationFunctionType
    ALU = mybir.AluOpType
    ts = bass.ts

    nc = bacc.Bacc(
        "TRN2",
        target_bir_lowering=False,
        debug=False,
        num_devices=NCORES,
    )

    xc = nc.declare_dram_parameter("xc", [CHUNK, DIM], fp32, isOutput=False)
    xf = nc.declare_dram_parameter("xf", [1, DIM], fp32, isOutput=False)
    W0 = nc.declare_dram_parameter("W0", [DIM, DIM], fp32, isOutput=False)
    W1 = nc.declare_dram_parameter("W1", [DIM, DIM], fp32, isOutput=False)
    W2 = nc.declare_dram_parameter("W2", [DIM, DIM], fp32, isOutput=False)
    if table_mode == "host":
        cosT = nc.declare_dram_parameter("cosT", [HALF, CHUNK], fp32, isOutput=False)
        sinT = nc.declare_dram_parameter("sinT", [HALF, CHUNK], fp32, isOutput=False)
    else:
        divT = nc.declare_dram_parameter("divT", [HALF, 1], fp32, isOutput=False)
        t0div = nc.declare_dram_parameter("t0div", [HALF, 1], fp32, isOutput=False)
    out_v = nc.declare_dram_parameter("out_v", [1, DIM], fp32, isOutput=True)
    out_m = nc.declare_dram_parameter("out_m", [1, 1], fp32, isOutput=True)
    out_s = nc.declare_dram_parameter("out_s", [1, 1], fp32, isOutput=True)

    with tile.TileContext(nc) as tc:
        with (
            tc.tile_pool(name="const", bufs=1) as const,
            tc.tile_pool(name="tmp", bufs=2) as tmp,
            tc.tile_pool(name="rr", bufs=2) as rrp,
            tc.tile_pool(name="ps", bufs=2, space="PSUM") as ps,
        ):
            # ---- constants / inputs into SBUF ----
            # DMA order matters: SP dispatch is ~500ns/DMA serial, so the
            # table-build inputs and W0/W1 (the long dependency chains) go
            # first, and everything is coalesced into few transfers.
            W0sb = const.tile([128, 2, DIM], fp32)
            W1sb = const.tile([128, 2, DIM], fp32)
            W2sb = const.tile([128, 2, DIM], fp32)
            xfT = const.tile([128, 2, 1], fp32)
            cos_sb = const.tile([HALF, CHUNK], fp32)
            sin_sb = const.tile([HALF, CHUNK], fp32)
            if table_mode == "device":
                div_sb = const.tile([HALF, 1], fp32)
                t0d_sb = const.tile([HALF, 1], fp32)
                nc.sync.dma_start(out=div_sb, in_=divT[:, :])
                nc.sync.dma_start(out=t0d_sb, in_=t0div[:, :])
            nc.sync.dma_start(
                out=xfT, in_=xf[0:1, :].rearrange("o (a p) -> p a o", p=128)
            )
            nc.sync.dma_start(out=W1sb, in_=W1.rearrange("(a p) h -> p a h", p=128))
            nc.sync.dma_start(out=W0sb, in_=W0.rearrange("(a p) h -> p a h", p=128))

            xn = const.tile([128, NJ, DIM], fp32)
            for jj in range(4):
                nc.sync.dma_start(
                    out=xn[:, ts(jj, 4), :],
                    in_=xc.rearrange("(j p) i -> p j i", p=128)[:, ts(jj, 4), :],
                )
            nc.sync.dma_start(out=W2sb, in_=W2.rearrange("(a p) h -> p a h", p=128))

            if table_mode == "host":
                nc.sync.dma_start(out=cos_sb, in_=cosT[:, :])
                nc.sync.dma_start(out=sin_sb, in_=sinT[:, :])
            else:
                # Per-512-chunk table build so chunk 0's sin/cos are ready
                # ~6us in and the score pipeline overlaps the rest.
                # Range-reduce ang by 2*pi*k (k = round(ang / 2pi)) with a
                # 3-term Cody-Waite cascade; k*c1 is exact in f32 here
                # (k <= 652 fits 10 bits, c1 has 9 significand bits).
                two_pi = 2.0 * math.pi
                c1 = float(np.float32(6.28125))
                c2 = float(np.float32(two_pi - 6.28125))
                c3 = float(np.float32(two_pi - c1 - np.float64(np.float32(two_pi - 6.28125))))
                inv2pi = float(np.float32(1.0 / two_pi))
                magic = 12582912.0  # 1.5 * 2**23: float32 round-to-nearest-int
                PF = float(np.float32(3.1415925))  # just below pi, sim-safe clamp

                zero_b = const.tile([HALF, 1], fp32)
                nc.vector.memset(zero_b, 0.0)

                for n in range(NCH):
                    csl = ts(n, 512)
                    titer = tmp.tile([HALF, 512], fp32, tag="titer")
                    nc.gpsimd.iota(
                        titer,
                        pattern=[[1, 512]],
                        base=n * 512,
                        channel_multiplier=0,
                        allow_small_or_imprecise_dtypes=True,
                    )
                    ang = tmp.tile([HALF, 512], fp32, tag="ang")
                    # ang[k, t'] = t' * div_k + t0 * div_k
                    nc.gpsimd.tensor_scalar(
                        out=ang, in0=titer, scalar1=div_sb, scalar2=t0d_sb,
                        op0=ALU.mult, op1=ALU.add,
                    )
                    # sin: k = round(ang * inv2pi); red = ang - k*2pi
                    u = tmp.tile([HALF, 512], fp32, tag="u")
                    k = tmp.tile([HALF, 512], fp32, tag="k")
                    red = tmp.tile([HALF, 512], fp32, tag="red")
                    nc.gpsimd.tensor_scalar(
                        out=u, in0=ang, scalar1=inv2pi, scalar2=magic,
                        op0=ALU.mult, op1=ALU.add,
                    )
                    nc.gpsimd.tensor_scalar_sub(k, u, magic)
                    nc.vector.cody_waite_cascade(red, ang, k, c1, c2, c3)
                    nc.vector.tensor_scalar(
                        out=red, in0=red, scalar1=-PF, scalar2=PF,
                        op0=ALU.max, op1=ALU.min,
                    )
                    nc.scalar.activation(
                        sin_sb[:, csl], red, AF.Sin, bias=zero_b, scale=1.0
                    )
                    # cos: k2 = round(ang*inv2pi + 0.25); red2 = ang - k2*2pi
                    # cos(ang) = Sin(red2 + pi/2), +pi/2 folded into the clamp
                    u2 = tmp.tile([HALF, 512], fp32, tag="u2")
                    k2 = tmp.tile([HALF, 512], fp32, tag="k2")
                    red2 = tmp.tile([HALF, 512], fp32, tag="red2")
                    nc.gpsimd.tensor_scalar(
                        out=u2, in0=ang, scalar1=inv2pi, scalar2=0.25,
                        op0=ALU.mult, op1=ALU.add,
                    )
                    nc.gpsimd.tensor_scalar(
                        out=k2, in0=u2, scalar1=magic, scalar2=magic,
                        op0=ALU.add, op1=ALU.subtract,
                    )
                    nc.vector.cody_waite_cascade(red2, ang, k2, c1, c2, c3)
                    nc.vector.tensor_scalar(
                        out=red2, in0=red2,
                        scalar1=float(np.float32(math.pi / 2.0)), scalar2=PF,
                        op0=ALU.add, op1=ALU.min,
                    )
                    nc.vector.tensor_scalar_max(red2, red2, -PF)
                    nc.scalar.activation(
                        cos_sb[:, csl], red2, AF.Sin, bias=zero_b, scale=1.0
                    )

            identity = const.tile([128, 128], fp32)
            make_identity(nc, identity)
            ones_r = const.tile([1, 128], fp32)   # row of ones (x0 broadcast)
            nc.vector.memset(ones_r, 1.0)
            ones_c = const.tile([128, 1], fp32)   # column of ones (k reduction)
            nc.vector.memset(ones_c, 1.0)
            ones_1 = const.tile([1, 1], fp32)     # scalar one (eT/gT transposes)
            nc.vector.memset(ones_1, 1.0)

            # ---- x0 = xf @ W0 ----
            x0_ps = ps.tile([1, DIM], fp32, tag="misc")
            nc.tensor.matmul(x0_ps, xfT[:, 0, :], W0sb[:, 0, :], start=True, stop=False)
            nc.tensor.matmul(x0_ps, xfT[:, 1, :], W0sb[:, 1, :], start=False, stop=True)
            x0_sb = const.tile([1, DIM], fp32)
            nc.vector.tensor_copy(x0_sb, x0_ps)

            # broadcast x0 to all 128 partitions: ones_r.T @ x0
            x0b_ps = ps.tile([128, DIM], fp32, tag="misc")
            nc.tensor.matmul(x0b_ps, ones_r, x0_sb, start=True, stop=True)
            x0b = const.tile([128, DIM], fp32)
            nc.vector.tensor_copy(x0b, x0b_ps)

            # ---- A | B (built straight into bf16 for the scores matmuls) ----
            bf16 = mybir.dt.bfloat16
            AB = const.tile([128, 2, DIM], bf16)  # [:, a, 0:128] = A, [:, a, 128:256] = B
            x0e = x0b.rearrange("p (k two) -> p two k", two=2)[:, 0:1, :]
            x0o = x0b.rearrange("p (k two) -> p two k", two=2)[:, 1:2, :]
            for a in range(2):
                W1e = W1sb[:, a, :].rearrange("p (k two) -> p two k", two=2)[:, 0:1, :]
                W1o = W1sb[:, a, :].rearrange("p (k two) -> p two k", two=2)[:, 1:2, :]
                t1 = tmp.tile([128, 128], fp32, tag="t1")
                t2 = tmp.tile([128, 128], fp32, tag="t2")
                nc.vector.tensor_mul(t1, W1e, x0e)
                nc.vector.tensor_mul(t2, W1o, x0o)
                nc.vector.tensor_add(AB[:, a, 0:128], t1, t2)
                t3 = tmp.tile([128, 128], fp32, tag="t3")
                t4 = tmp.tile([128, 128], fp32, tag="t4")
                nc.vector.tensor_mul(t3, W1e, x0o)
                nc.vector.tensor_mul(t4, W1o, x0e)
                nc.vector.tensor_sub(AB[:, a, 128:256], t3, t4)

            # ---- transpose x: xT[a][i, t], downcast to bf16 in the copy ----
            xT = const.tile([128, 2, CHUNK], bf16)
            for j in range(NJ):
                for a in range(2):
                    tp_ps = ps.tile([128, 128], fp32, tag="tp")
                    nc.tensor.transpose(tp_ps, xn[:, j, ts(a, 128)], identity)
                    if (2 * j + a) % 2 == 1:
                        nc.scalar.copy(xT[:, a, ts(j, 128)], tp_ps)
                    else:
                        nc.vector.tensor_copy(xT[:, a, ts(j, 128)], tp_ps)

            # ---- scores ----
            sc_sb = const.tile([1, CHUNK], fp32)
            m4 = const.tile([1, NCH], fp32)
            for n in range(NCH):
                csl = ts(n, 512)
                p_ps = ps.tile([128, 512], fp32, tag="p")
                nc.tensor.matmul(p_ps, AB[:, 0, 0:128], xT[:, 0, csl], start=True, stop=False)
                nc.tensor.matmul(p_ps, AB[:, 1, 0:128], xT[:, 1, csl], start=False, stop=True)
                q_ps = ps.tile([128, 512], fp32, tag="q")
                nc.tensor.matmul(q_ps, AB[:, 0, 128:256], xT[:, 0, csl], start=True, stop=False)
                nc.tensor.matmul(q_ps, AB[:, 1, 128:256], xT[:, 1, csl], start=False, stop=True)
                rp = rrp.tile([128, 512], fp32, tag="rp")
                rq = rrp.tile([128, 512], fp32, tag="rq")
                rr = rrp.tile([128, 512], fp32, tag="rr")
                nc.vector.tensor_mul(rp, cos_sb[:, csl], p_ps)
                nc.vector.tensor_mul(rq, sin_sb[:, csl], q_ps)
                nc.gpsimd.tensor_add(rr, rp, rq)
                sc_ps = ps.tile([1, 512], fp32, tag="misc")
                nc.tensor.matmul(sc_ps, ones_c, rr, start=True, stop=True)
                nc.scalar.copy(sc_sb[:, csl], sc_ps)
                nc.vector.reduce_max(
                    m4[:, n : n + 1], sc_ps, axis=mybir.AxisListType.X
                )

            # ---- softmax pieces (chunked so eT/g can start early) ----
            m_sb = const.tile([1, 1], fp32)
            nc.vector.reduce_max(m_sb, m4, axis=mybir.AxisListType.X)
            negm = const.tile([1, 1], fp32)
            nc.vector.tensor_scalar_mul(negm, m_sb, -1.0)
            e_sb = const.tile([1, CHUNK], fp32)
            s4 = const.tile([1, NCH], fp32)
            s_sb = const.tile([1, 1], fp32)
            eT = const.tile([128, NJ], fp32)
            for n in range(NCH):
                csl = ts(n, 512)
                nc.scalar.activation(
                    e_sb[:, csl], sc_sb[:, csl], AF.Exp, bias=negm, scale=1.0,
                    accum_out=s4[:, n : n + 1],
                )
                # eT: e with t on partitions, 4 row-tiles per chunk
                for j in range(4 * n, 4 * n + 4):
                    et_ps = ps.tile([128, 1], fp32, tag="tp")
                    nc.tensor.matmul(
                        et_ps, e_sb[:, ts(j, 128)], ones_1, start=True, stop=True
                    )
                    nc.vector.tensor_copy(eT[:, j : j + 1], et_ps)
            nc.vector.reduce_sum(s_sb, s4, axis=mybir.AxisListType.X)

            # ---- g = e^T @ x  (1, 256) ----
            g_ps = ps.tile([1, DIM], fp32, tag="misc")
            for j in range(NJ):
                nc.tensor.matmul(
                    g_ps, eT[:, j : j + 1], xn[:, j, :],
                    start=(j == 0), stop=(j == NJ - 1),
                )
            g_sb = const.tile([1, DIM], fp32)
            nc.vector.tensor_copy(g_sb, g_ps)

            # ---- v = g @ W2 ----
            gT = const.tile([128, 2], fp32)
            for a in range(2):
                gt_ps = ps.tile([128, 1], fp32, tag="tp")
                nc.tensor.matmul(gt_ps, g_sb[:, ts(a, 128)], ones_1, start=True, stop=True)
                nc.vector.tensor_copy(gT[:, a : a + 1], gt_ps)
            v_ps = ps.tile([1, DIM], fp32, tag="misc")
            for a in range(2):
                nc.tensor.matmul(
                    v_ps, gT[:, a : a + 1], W2sb[:, a, :],
                    start=(a == 0), stop=(a == 1),
                )
            v_sb = const.tile([1, DIM], fp32)
            nc.vector.tensor_copy(v_sb, v_ps)

            nc.sync.dma_start(out=out_v[:, :], in_=v_sb)
            nc.sync.dma_start(out=out_m[:, :], in_=m_sb)
            nc.sync.dma_start(out=out_s[:, :], in_=s_sb)

    nc.finalize()
    return nc


def get_nc():
    key = ("nc", TABLE_MODE)
    if key not in _CACHE:
        _CACHE[key] = _build_nc(TABLE_MODE)
    return _CACHE[key]


def _div_f32():
    # matches reference: exp(arange(half) * (-log(10000.)/half)) in float32
    scale = np.float32(-np.log(np.float32(10000.0)) / np.float32(HALF))
    return np.exp(np.arange(HALF, dtype=np.float32) * scale).astype(np.float32)


def make_in_maps(x, W0, W1, W2):
    div = _div_f32()
    in_maps = []
    for c in range(NCORES):
        b, h = c // 2, c % 2
        t0 = h * CHUNK
        im = {
            "xc": np.ascontiguousarray(x[b, t0 : t0 + CHUNK, :]),
            "xf": np.ascontiguousarray(x[b, 0:1, :]),
            "W0": W0,
            "W1": W1,
            "W2": W2,
        }
        if TABLE_MODE == "host":
            t = np.arange(t0, t0 + CHUNK, dtype=np.float32)
            ang = (div[:, None] * t[None, :]).astype(np.float32)
            im["cosT"] = np.cos(ang.astype(np.float64)).astype(np.float32)
            im["sinT"] = np.sin(ang.astype(np.float64)).astype(np.float32)
        else:
            im["divT"] = div.reshape(HALF, 1)
            im["t0div"] = (np.float32(t0) * div).astype(np.float32).reshape(HALF, 1)
        in_maps.append(im)
    return in_maps


def combine(results):
    out = np.zeros((NB, DIM), dtype=np.float32)
    for b in range(NB):
        r0 = results[2 * b]
        r1 = results[2 * b + 1]
        m0 = float(r0["out_m"][0, 0]); s0 = float(r0["out_s"][0, 0])
        m1 = float(r1["out_m"][0, 0]); s1 = float(r1["out_s"][0, 0])
        v0 = r0["out_v"][0].astype(np.float64)
        v1 = r1["out_v"][0].astype(np.float64)
        M = max(m0, m1)
        w0 = math.exp(m0 - M); w1 = math.exp(m1 - M)
        V = v0 * w0 + v1 * w1
        S = s0 * w0 + s1 * w1
        out[b] = (V / S).astype(np.float32)
    return out


def kernel(**inputs):
    from concourse.bass_utils import run_bass_kernel_spmd

    x = np.asarray(inputs["x"], dtype=np.float32)
    W0 = np.ascontiguousarray(np.asarray(inputs["W0"], dtype=np.float32))
    W1 = np.ascontiguousarray(np.asarray(inputs["W1"], dtype=np.float32))
    W2 = np.ascontiguousarray(np.asarray(inputs["W2"], dtype=np.float32))

    nc = get_nc()
    in_maps = make_in_maps(x, W0, W1, W2)
    res = run_bass_kernel_spmd(nc, in_maps, core_ids=list(range(NCORES)))
    return combine(res.results)


# revision 24
# speedup vs baseline: 1.0195x; 1.0195x over previous
"""Trainium2 Bass kernel for nn_CustomModel_31361851195525.

Reference computation (b=4, s=4096, d=256):
    x0 = rope(x @ W0)[:, 0, :]          # rope at pos 0 == identity
    x1 = rope(x @ W1)                   # (b, s, d)
    scores[b, t] = x0[b] . x1[b, t]     # only row 0 of the score matrix is used
    attn = softmax(scores)              # (b, s)
    out[b] = sum_t attn[b, t] * (x @ W2)[b, t]
           = (attn[b] @ x[b]) @ W2      # reassociated: kills the big x@W2 matmul

Score folding (avoids materializing rope(x@W1)):
    scores[t] = sum_k cos[t,k] * (x_t . A_k) + sin[t,k] * (x_t . B_k)
    A[:, k] = W1[:, 2k] * x0[2k]   + W1[:, 2k+1] * x0[2k+1]
    B[:, k] = W1[:, 2k] * x0[2k+1] - W1[:, 2k+1] * x0[2k]

Sharding: core c -> batch c//2, sequence half c%2 (2048 rows each).
Each core returns (v, m, s) = (unnormalized weighted x-sum @ W2, chunk score
max, chunk exp-sum); the host combines the two chunks per batch with the
standard log-sum-exp merge.
"""

import math

import numpy as np

SEQ = 4096
DIM = 256
HALF = 128
NB = 4
NCORES = 8
CHUNK = SEQ // 2       # rows per core
NJ = CHUNK // 128      # 16 row tiles per core
NCH = CHUNK // 512     # 4 score chunks per core

# "host": sin/cos tables computed on host and DMA'd in (2MB extra per core).
# "device": tables built on-device via iota + mod range-reduction + Sin LUT.
TABLE_MODE = "device"

_CACHE = {}


def _build_nc(table_mode):
    import concourse.bass as bass
    import concourse.mybir as mybir
    import concourse.tile as tile
    from concourse import bacc
    from concourse.masks import make_identity

    fp32 = mybir.dt.float32
    AF = mybir.ActivationFunctionType
    ALU = mybir.AluOpType
    ts = bass.ts

    nc = bacc.Bacc(
        "TRN2",
        target_bir_lowering=False,
        debug=False,
        num_devices=NCORES,
    )

    xc = nc.declare_dram_parameter("xc", [CHUNK, DIM], fp32, isOutput=False)
    xf = nc.declare_dram_parameter("xf", [1, DIM], fp32, isOutput=False)
    W0 = nc.declare_dram_parameter("W0", [DIM, DIM], fp32, isOutput=False)
    W1 = nc.declare_dram_parameter("W1", [DIM, DIM], fp32, isOutput=False)
    W2 = nc.declare_dram_parameter("W2", [DIM, DIM], fp32, isOutput=False)
    if table_mode == "host":
        cosT = nc.declare_dram_parameter("cosT", [HALF, CHUNK], fp32, isOutput=False)
        sinT = nc.declare_dram_parameter("sinT", [HALF, CHUNK], fp32, isOutput=False)
    else:
        divT = nc.declare_dram_parameter("divT", [HALF, 1], fp32, isOutput=False)
        t0div = nc.declare_dram_parameter("t0div", [HALF, 1], fp32, isOutput=False)
    out_v = nc.declare_dram_parameter("out_v", [1, DIM], fp32, isOutput=True)
    out_m = nc.declare_dram_parameter("out_m", [1, 1], fp32, isOutput=True)
    out_s = nc.declare_dram_parameter("out_s", [1, 1], fp32, isOutput=True)

    with tile.TileContext(nc) as tc:
        with (
            tc.tile_pool(name="const", bufs=1) as const,
            tc.tile_pool(name="tmp", bufs=3) as tmp,
            tc.tile_pool(name="rr", bufs=3) as rrp,
            tc.tile_pool(name="ps", bufs=2, space="PSUM") as ps,
        ):
            # ---- constants / inputs into SBUF ----
            # DMA order matters: SP dispatch is ~500ns/DMA serial, so the
            # table-build inputs and W0/W1 (the long dependency chains) go
            # first, and everything is coalesced into few transfers.
            W0sb = const.tile([128, 2, DIM], fp32)
            W1sb = const.tile([128, 2, DIM], fp32)
            W2sb = const.tile([128, 2, DIM], fp32)
            xfT = const.tile([128, 2, 1], fp32)
            cos_sb = const.tile([HALF, CHUNK], fp32)
            sin_sb = const.tile([HALF, CHUNK], fp32)
            if table_mode == "device":
                div_sb = const.tile([HALF, 1], fp32)
                t0d_sb = const.tile([HALF, 1], fp32)
                nc.sync.dma_start(out=div_sb, in_=divT[:, :])
                nc.sync.dma_start(out=t0d_sb, in_=t0div[:, :])
            nc.sync.dma_start(
                out=xfT, in_=xf[0:1, :].rearrange("o (a p) -> p a o", p=128)
            )
            nc.sync.dma_start(out=W1sb, in_=W1.rearrange("(a p) h -> p a h", p=128))
            nc.sync.dma_start(out=W0sb, in_=W0.rearrange("(a p) h -> p a h", p=128))

            xn = const.tile([128, NJ, DIM], fp32)
            for jj in range(4):
                nc.sync.dma_start(
                    out=xn[:, ts(jj, 4), :],
                    in_=xc.rearrange("(j p) i -> p j i", p=128)[:, ts(jj, 4), :],
                )
            nc.sync.dma_start(out=W2sb, in_=W2.rearrange("(a p) h -> p a h", p=128))

            if table_mode == "host":
                nc.sync.dma_start(out=cos_sb, in_=cosT[:, :])
                nc.sync.dma_start(out=sin_sb, in_=sinT[:, :])
            else:
                # Per-512-chunk table build so chunk 0's sin/cos are ready
                # ~6us in and the score pipeline overlaps the rest.
                # Range-reduce ang by 2*pi*k (k = round(ang / 2pi)) with a
                # 3-term Cody-Waite cascade; k*c1 is exact in f32 here
                # (k <= 652 fits 10 bits, c1 has 9 significand bits).
                two_pi = 2.0 * math.pi
                c1 = float(np.float32(6.28125))
                c2 = float(np.float32(two_pi - 6.28125))
                c3 = float(np.float32(two_pi - c1 - np.float64(np.float32(two_pi - 6.28125))))
                inv2pi = float(np.float32(1.0 / two_pi))
                magic = 12582912.0  # 1.5 * 2**23: float32 round-to-nearest-int
                PF = float(np.float32(3.1415925))  # just below pi, sim-safe clamp

                zero_b = const.tile([HALF, 1], fp32)
                nc.vector.memset(zero_b, 0.0)

                for n in range(NCH):
                    csl = ts(n, 512)
                    titer = tmp.tile([HALF, 512], fp32, tag="titer")
                    nc.gpsimd.iota(
                        titer,
                        pattern=[[1, 512]],
                        base=n * 512,
                        channel_multiplier=0,
                        allow_small_or_imprecise_dtypes=True,
                    )
                    ang = tmp.tile([HALF, 512], fp32, tag="ang")
                    # ang[k, t'] = t' * div_k + t0 * div_k
                    nc.gpsimd.tensor_scalar(
                        out=ang, in0=titer, scalar1=div_sb, scalar2=t0d_sb,
                        op0=ALU.mult, op1=ALU.add,
                    )
                    # sin: k = round(ang * inv2pi); red = ang - k*2pi
                    u = tmp.tile([HALF, 512], fp32, tag="u")
                    k = tmp.tile([HALF, 512], fp32, tag="k")
                    red = tmp.tile([HALF, 512], fp32, tag="red")
                    nc.gpsimd.tensor_scalar(
                        out=u, in0=ang, scalar1=inv2pi, scalar2=magic,
                        op0=ALU.mult, op1=ALU.add,
                    )
                    nc.gpsimd.tensor_scalar_sub(k, u, magic)
                    nc.vector.cody_waite_cascade(red, ang, k, c1, c2, c3)
                    nc.vector.tensor_scalar(
                        out=red, in0=red, scalar1=-PF, scalar2=PF,
                        op0=ALU.max, op1=ALU.min,
                    )
                    nc.scalar.activation(
                        sin_sb[:, csl], red, AF.Sin, bias=zero_b, scale=1.0
                    )
                    # cos: k2 = round(ang*inv2pi + 0.25); red2 = ang - k2*2pi
                    # cos(ang) = Sin(red2 + pi/2), +pi/2 folded into the clamp
                    u2 = tmp.tile([HALF, 512], fp32, tag="u2")
                    k2 = tmp.tile([HALF, 512], fp32, tag="k2")
                    red2 = tmp.tile([HALF, 512], fp32, tag="red2")
                    nc.gpsimd.tensor_scalar(
                        out=u2, in0=ang, scalar1=inv2pi, scalar2=0.25,
                        op0=ALU.mult, op1=ALU.add,
                    )
                    nc.gpsimd.tensor_scalar(
                        out=k2, in0=u2, scalar1=magic, scalar2=magic,
                        op0=ALU.add, op1=ALU.subtract,
                    )
                    nc.vector.cody_waite_cascade(red2, ang, k2, c1, c2, c3)
                    nc.vector.tensor_scalar(
                        out=red2, in0=red2,
                        scalar1=float(np.float32(math.pi / 2.0)), scalar2=PF,
                        op0=ALU.add, op1=ALU.min,
                    )
                    nc.vector.tensor_scalar_max(red2, red2, -PF)
                    nc.scalar.activation(
                        cos_sb[:, csl], red2, AF.Sin, bias=zero_b, scale=1.0
                    )

            identity = const.tile([128, 128], fp32)
            make_identity(nc, identity)
            ones_r = const.tile([1, 128], fp32)   # row of ones (x0 broadcast)
            nc.vector.memset(ones_r, 1.0)
            ones_c = const.tile([128, 1], fp32)   # column of ones (k reduction)
            nc.vector.memset(ones_c, 1.0)
            ones_1 = const.tile([1, 1], fp32)     # scalar one (eT/gT transposes)
            nc.vector.memset(ones_1, 1.0)

            # ---- x0 = xf @ W0 ----
            x0_ps = ps.tile([1, DIM], fp32, tag="misc")
            nc.tensor.matmul(x0_ps, xfT[:, 0, :], W0sb[:, 0, :], start=True, stop=False)
            nc.tensor.matmul(x0_ps, xfT[:, 1, :], W0sb[:, 1, :], start=False, stop=True)
            x0_sb = const.tile([1, DIM], fp32)
            nc.vector.tensor_copy(x0_sb, x0_ps)

            # broadcast x0 to all 128 partitions: ones_r.T @ x0
            x0b_ps = ps.tile([128, DIM], fp32, tag="misc")
            nc.tensor.matmul(x0b_ps, ones_r, x0_sb, start=True, stop=True)
            x0b = const.tile([128, DIM], fp32)
            nc.vector.tensor_copy(x0b, x0b_ps)

            # ---- A | B (built straight into bf16 for the scores matmuls) ----
            bf16 = mybir.dt.bfloat16
            AB = const.tile([128, 2, DIM], bf16)  # [:, a, 0:128] = A, [:, a, 128:256] = B
            x0e = x0b.rearrange("p (k two) -> p two k", two=2)[:, 0:1, :]
            x0o = x0b.rearrange("p (k two) -> p two k", two=2)[:, 1:2, :]
            for a in range(2):
                W1e = W1sb[:, a, :].rearrange("p (k two) -> p two k", two=2)[:, 0:1, :]
                W1o = W1sb[:, a, :].rearrange("p (k two) -> p two k", two=2)[:, 1:2, :]
                t1 = tmp.tile([128, 128], fp32, tag="t1")
                t2 = tmp.tile([128, 128], fp32, tag="t2")
                nc.vector.tensor_mul(t1, W1e, x0e)
                nc.vector.tensor_mul(t2, W1o, x0o)
                nc.vector.tensor_add(AB[:, a, 0:128], t1, t2)
                t3 = tmp.tile([128, 128], fp32, tag="t3")
                t4 = tmp.tile([128, 128], fp32, tag="t4")
                nc.vector.tensor_mul(t3, W1e, x0o)
                nc.vector.tensor_mul(t4, W1o, x0e)
                nc.vector.tensor_sub(AB[:, a, 128:256], t3, t4)

            # ---- transpose x: xT[a][i, t], downcast to bf16 in the copy ----
            xT = const.tile([128, 2, CHUNK], bf16)
            for j in range(NJ):
                for a in range(2):
                    tp_ps = ps.tile([128, 128], fp32, tag="tp")
                    nc.tensor.transpose(tp_ps, xn[:, j, ts(a, 128)], identity)
                    if (2 * j + a) % 2 == 1:
                        nc.scalar.copy(xT[:, a, ts(j, 128)], tp_ps)
                    else:
                        nc.vector.tensor_copy(xT[:, a, ts(j, 128)], tp_ps)

            # ---- scores ----
            sc_sb = const.tile([1, CHUNK], fp32)
            m4 = const.tile([1, NCH], fp32)
            for n in range(NCH):
                csl = ts(n, 512)
                p_ps = ps.tile([128, 512], fp32, tag="p")
                nc.tensor.matmul(p_ps, AB[:, 0, 0:128], xT[:, 0, csl], start=True, stop=False)
                nc.tensor.matmul(p_ps, AB[:, 1, 0:128], xT[:, 1, csl], start=False, stop=True)
                q_ps = ps.tile([128, 512], fp32, tag="q")
                nc.tensor.matmul(q_ps, AB[:, 0, 128:256], xT[:, 0, csl], start=True, stop=False)
                nc.tensor.matmul(q_ps, AB[:, 1, 128:256], xT[:, 1, csl], start=False, stop=True)
                rp = rrp.tile([128, 512], fp32, tag="rp")
                rq = rrp.tile([128, 512], fp32, tag="rq")
                rr = rrp.tile([128, 512], fp32, tag="rr")
                nc.vector.tensor_mul(rp, cos_sb[:, csl], p_ps)
                nc.vector.tensor_mul(rq, sin_sb[:, csl], q_ps)
                nc.gpsimd.tensor_add(rr, rp, rq)
                sc_ps = ps.tile([1, 512], fp32, tag="misc")
                nc.tensor.matmul(sc_ps, ones_c, rr, start=True, stop=True)
                nc.scalar.copy(sc_sb[:, csl], sc_ps)
                nc.vector.reduce_max(
                    m4[:, n : n + 1], sc_ps, axis=mybir.AxisListType.X
                )

            # ---- softmax pieces (chunked so eT/g can start early) ----
            m_sb = const.tile([1, 1], fp32)
            nc.vector.reduce_max(m_sb, m4, axis=mybir.AxisListType.X)
            negm = const.tile([1, 1], fp32)
            nc.vector.tensor_scalar_mul(negm, m_sb, -1.0)
            e_sb = const.tile([1, CHUNK], fp32)
            s4 = const.tile([1, NCH], fp32)
            s_sb = const.tile([1, 1], fp32)
            eT = const.tile([128, NJ], fp32)
            for n in range(NCH):
                csl = ts(n, 512)
                nc.scalar.activation(
                    e_sb[:, csl], sc_sb[:, csl], AF.Exp, bias=negm, scale=1.0,
                    accum_out=s4[:, n : n + 1],
                )
                # eT: e with t on partitions, 4 row-tiles per chunk
                for j in range(4 * n, 4 * n + 4):
                    et_ps = ps.tile([128, 1], fp32, tag="tp")
                    nc.tensor.matmul(
                        et_ps, e_sb[:, ts(j, 128)], ones_1, start=True, stop=True
                    )
                    nc.vector.tensor_copy(eT[:, j : j + 1], et_ps)
            nc.vector.reduce_sum(s_sb, s4, axis=mybir.AxisListType.X)

            # ---- g = e^T @ x  (1, 256) ----
            g_ps = ps.tile([1, DIM], fp32, tag="misc")
            for j in range(NJ):
                nc.tensor.matmul(
                    g_ps, eT[:, j : j + 1], xn[:, j, :],
                    start=(j == 0), stop=(j == NJ - 1),
                )
            g_sb = const.tile([1, DIM], fp32)
            nc.vector.tensor_copy(g_sb, g_ps)

            # ---- v = g @ W2 ----
            gT = const.tile([128, 2], fp32)
            for a in range(2):
                gt_ps = ps.tile([128, 1], fp32, tag="tp")
                nc.tensor.matmul(gt_ps, g_sb[:, ts(a, 128)], ones_1, start=True, stop=True)
                nc.vector.tensor_copy(gT[:, a : a + 1], gt_ps)
            v_ps = ps.tile([1, DIM], fp32, tag="misc")
            for a in range(2):
                nc.tensor.matmul(
                    v_ps, gT[:, a : a + 1], W2sb[:, a, :],
                    start=(a == 0), stop=(a == 1),
                )
            v_sb = const.tile([1, DIM], fp32)
            nc.vector.tensor_copy(v_sb, v_ps)

            nc.sync.dma_start(out=out_v[:, :], in_=v_sb)
            nc.sync.dma_start(out=out_m[:, :], in_=m_sb)
            nc.sync.dma_start(out=out_s[:, :], in_=s_sb)

    nc.finalize()
    return nc


def get_nc():
    key = ("nc", TABLE_MODE)
    if key not in _CACHE:
        _CACHE[key] = _build_nc(TABLE_MODE)
    return _CACHE[key]


def _div_f32():
    # matches reference: exp(arange(half) * (-log(10000.)/half)) in float32
    scale = np.float32(-np.log(np.float32(10000.0)) / np.float32(HALF))
    return np.exp(np.arange(HALF, dtype=np.float32) * scale).astype(np.float32)


def make_in_maps(x, W0, W1, W2):
    div = _div_f32()
    in_maps = []
    for c in range(NCORES):
        b, h = c // 2, c % 2
        t0 = h * CHUNK
        im = {
            "xc": np.ascontiguousarray(x[b, t0 : t0 + CHUNK, :]),
            "xf": np.ascontiguousarray(x[b, 0:1, :]),
            "W0": W0,
            "W1": W1,
            "W2": W2,
        }
        if TABLE_MODE == "host":
            t = np.arange(t0, t0 + CHUNK, dtype=np.float32)
            ang = (div[:, None] * t[None, :]).astype(np.float32)
            im["cosT"] = np.cos(ang.astype(np.float64)).astype(np.float32)
            im["sinT"] = np.sin(ang.astype(np.float64)).astype(np.float32)
        else:
            im["divT"] = div.reshape(HALF, 1)
            im["t0div"] = (np.float32(t0) * div).astype(np.float32).reshape(HALF, 1)
        in_maps.append(im)
    return in_maps


def combine(results):
    out = np.zeros((NB, DIM), dtype=np.float32)
    for b in range(NB):
        r0 = results[2 * b]
        r1 = results[2 * b + 1]
        m0 = float(r0["out_m"][0, 0]); s0 = float(r0["out_s"][0, 0])
        m1 = float(r1["out_m"][0, 0]); s1 = float(r1["out_s"][0, 0])
        v0 = r0["out_v"][0].astype(np.float64)
        v1 = r1["out_v"][0].astype(np.float64)
        M = max(m0, m1)
        w0 = math.exp(m0 - M); w1 = math.exp(m1 - M)
        V = v0 * w0 + v1 * w1
        S = s0 * w0 + s1 * w1
        out[b] = (V / S).astype(np.float32)
    return out


def kernel(**inputs):
    from concourse.bass_utils import run_bass_kernel_spmd

    x = np.asarray(inputs["x"], dtype=np.float32)
    W0 = np.ascontiguousarray(np.asarray(inputs["W0"], dtype=np.float32))
    W1 = np.ascontiguousarray(np.asarray(inputs["W1"], dtype=np.float32))
    W2 = np.ascontiguousarray(np.asarray(inputs["W2"], dtype=np.float32))

    nc = get_nc()
    in_maps = make_in_maps(x, W0, W1, W2)
    res = run_bass_kernel_spmd(nc, in_maps, core_ids=list(range(NCORES)))
    return combine(res.results)
